# revision 11
# baseline (speedup 1.0000x reference)
"""Trainium2 Bass kernel for 3x3 conv (stride 1, pad 1) + bias.

x [32, 64, 224, 224] f32, weight [128, 64, 3, 3] f32, bias [128] f32
-> out [32, 128, 224, 224] f32.

Data-parallel over 8 NeuronCores: core c computes samples [4c, 4c+4).

Per-core scheme (v6, all dims hardcoded):
- Inputs cast to bf16 on host (PSUM accumulation stays fp32; rel err
  ~3e-3, well inside the gate). bf16 also enables FWL fast weight load.
- Even/odd row-parity packing, built on host with NO duplication:
  xeo[:, 0:64, r, :] = Ppad[ic, 2r, :], xeo[:, 64:128, r, :] =
  Ppad[ic, 2r+1, :]. Input HBM traffic is 1x (26.8 MB/core), half of
  the v4/v5 row-pair layout.
- Even output rows 2m need padded rows (2m, 2m+1, 2m+2) = (top slot m,
  bottom slot m, top slot m+1): one K=128 matmul @ slot m covers
  kh0(top)+kh1(bottom); the kh2 leftover is a K=64 top-half matmul @
  slot m+1. Odd rows 2m+1 need (2m+1, 2m+2, 2m+3) = (bottom m, top
  m+1, bottom m+1): one K=128 matmul @ slot m+1 covers kh1(top)+
  kh2(bottom); the kh0 leftover is a K=64 bottom-half matmul @ slot m.
  The even-row leftovers run on PE row-group 0-63 (tile_position
  (0,0)) and the odd-row leftovers on rows 64-127 ((64,0)), issued
  adjacently so each pair executes CONCURRENTLY: per 4 output rows,
  6 full-array + 3 concurrent-pair slots = the 4.5-slot/2-rows compute
  roofline (~376us/core intrinsic at 2.4 GHz).
- Each psum bank holds 2 same-parity rows; ScalarE evacuates with the
  fused bias add into the interleaved ot tile (bf16), so store DMAs
  write 8 contiguous output rows.
- Strips of 112 output rows = 57 even/odd slots, triple buffered;
  strip 0 lives in a dedicated tile reloaded at each For_i iteration
  tail so the body never starts with a DMA wait.
"""
import numpy as np
import ml_dtypes

import concourse.bass as bass
import concourse.mybir as mybir
import concourse.tile as tile
from concourse import bacc
from concourse.bass_utils import run_bass_kernel_spmd
from concourse._compat import axon_active

N_CORES = 8
S = 4                 # samples per core
IC, OC, H, W = 64, 128, 224, 224
HP, WP = H + 2, W + 2  # padded input dims (226)
XR = HP // 2          # 113 even/odd slot rows in HBM
QROWS = 112           # output rows per strip
SLOTS = QROWS // 2 + 1  # 57 slots per strip
NQ = H // QROWS       # 2 strips per sample
BLK = 2               # output rows per psum bank (same parity)
OBLK = 8              # output rows per psum group (4 banks)
SBLK = 16             # output rows per store tile (2 psum groups)

BF16 = mybir.dt.bfloat16
F32 = mybir.dt.float32
NPBF16 = ml_dtypes.bfloat16


def build_module(repeat=1):
    nc = bacc.Bacc("TRN2", target_bir_lowering=False, debug=not axon_active(),
                   enable_asserts=True, num_devices=N_CORES)
    # xeo[s, 0:64, r, c] = Ppad[ic, 2r, c]; [64:128] = Ppad[ic, 2r+1, c]
    xeo = nc.dram_tensor("xeo", [S, 2 * IC, XR, WP], BF16,
                         kind="ExternalInput").ap()
    # weven[0:64, kw*128+oc] = w[oc, ic, kh=0, kw]; [64:128] = kh=1
    weven = nc.dram_tensor("weven", [2 * IC, 3 * OC], BF16,
                           kind="ExternalInput").ap()
    # wodd: top = kh=1, bottom = kh=2
    wodd = nc.dram_tensor("wodd", [2 * IC, 3 * OC], BF16,
                          kind="ExternalInput").ap()
    # wleft: top = kh=2 (even-row leftover), bottom = kh=0 (odd leftover)
    wleft = nc.dram_tensor("wleft", [2 * IC, 3 * OC], BF16,
                           kind="ExternalInput").ap()
    bias = nc.dram_tensor("bias", [OC, 1], F32, kind="ExternalInput").ap()
    out = nc.dram_tensor("out", [S, OC, H, W], BF16, kind="ExternalOutput").ap()

    NT = S * NQ  # 16 strips, flattened (s, q)

    with tile.TileContext(nc) as tc:
        with tc.tile_pool(name="wp", bufs=1) as wp, \
             tc.tile_pool(name="s0p", bufs=1) as s0p, \
             tc.tile_pool(name="xp", bufs=3) as xp, \
             tc.tile_pool(name="op", bufs=4) as op, \
             tc.tile_pool(name="pp", bufs=2, space="PSUM") as pp:
            wet = wp.tile([2 * IC, 3 * OC], BF16)
            wot = wp.tile([2 * IC, 3 * OC], BF16)
            wlt = wp.tile([2 * IC, 3 * OC], BF16)
            btile = wp.tile([OC, 1], F32)
            nc.sync.dma_start(out=wet, in_=weven)
            nc.sync.dma_start(out=wot, in_=wodd)
            nc.sync.dma_start(out=wlt, in_=wleft)
            nc.sync.dma_start(out=btile, in_=bias)

            def load_strip_into(dst, t):
                s, q = divmod(t, NQ)
                nc.scalar.dma_start(
                    out=dst,
                    in_=xeo[s, :, q * (QROWS // 2):q * (QROWS // 2) + SLOTS, :])

            def load_strip(t):
                strip = xp.tile([2 * IC, SLOTS * WP], BF16, tag="strip")
                sr = strip.rearrange("p (r c) -> p r c", c=WP)
                load_strip_into(sr, t)
                return sr

            # Strip 0 lives in a dedicated tile, loaded once before the
            # repeat loop; each iteration's tail re-loads it for the next
            # iteration (For_i's all-engine barrier guarantees completion).
            s0t = s0p.tile([2 * IC, SLOTS * WP], BF16)
            s0r = s0t.rearrange("p (r c) -> p r c", c=WP)
            load_strip_into(s0r, 0)

            def compute(wrap):
                strips = {0: s0r}
                for t in range(NT):
                    s, q = divmod(t, NQ)
                    if t + 1 < NT:
                        strips[t + 1] = load_strip(t + 1)
                    elif wrap:
                        load_strip_into(s0r, 0)
                    sr = strips[t]

                    for g in range(QROWS // OBLK):
                        if g % 2 == 0:
                            ot = op.tile([OC, SBLK, W], BF16)
                            otv = ot.rearrange("p (m e) c -> p m e c", e=2)
                        ho = (g % 2) * (OBLK // BLK)  # ot half offset (m)
                        psums = [pp.tile([OC, BLK, W], F32, name="ps%d" % bb)
                                 for bb in range(4)]
                        # psum bb: 0=E0 rows(0,2) 1=O0 rows(1,3)
                        #          2=E1 rows(4,6) 3=O1 rows(5,7)
                        # E_k full @ slots (me, me+1); O_k @ (me+1, me+2)
                        # with me = 4g + 2k. Order E0,E1,O0,O1 so
                        # consecutive matmuls share stationary weights.
                        for kw in range(3):
                            for bb in (0, 2, 1, 3):
                                k, odd = divmod(bb, 2)
                                me = 4 * g + 2 * k + odd
                                lhsT = (wot if odd else wet)[
                                    :, kw * OC:(kw + 1) * OC]
                                nc.tensor.matmul(
                                    psums[bb], lhsT,
                                    sr[:, me:me + 2, kw:kw + W],
                                    start=(kw == 0), stop=False,
                                    skip_group_check=True)
                        # Leftovers: even kh2 on rows 0-63 @ slots
                        # (me+1, me+2); odd kh0 on rows 64-127 @ slots
                        # (me, me+1). A/B adjacent -> concurrent.
                        for kw in range(3):
                            for bb in range(4):
                                k, odd = divmod(bb, 2)
                                me = 4 * g + 2 * k
                                if odd:
                                    lhsT = wlt[IC:2 * IC,
                                               kw * OC:(kw + 1) * OC]
                                    rhs = sr[IC:2 * IC, me:me + 2,
                                             kw:kw + W]
                                else:
                                    lhsT = wlt[0:IC, kw * OC:(kw + 1) * OC]
                                    rhs = sr[0:IC, me + 1:me + 3,
                                             kw:kw + W]
                                nc.tensor.matmul(
                                    psums[bb], lhsT, rhs,
                                    start=False, stop=(kw == 2),
                                    skip_group_check=True)
                        for bb in range(4):
                            k, odd = divmod(bb, 2)
                            nc.scalar.activation(
                                otv[:, ho + 2 * k:ho + 2 * k + 2, odd, :],
                                psums[bb],
                                mybir.ActivationFunctionType.Identity,
                                bias=btile)
                        if g % 2 == 1:
                            oh0 = q * QROWS + (g - 1) * OBLK
                            nc.sync.dma_start(
                                out=out[s, :, oh0:oh0 + SBLK, :], in_=ot)

            if repeat == 1:
                compute(wrap=False)
            else:
                with tc.For_i(0, repeat, 1, staggered_reset=True):
                    compute(wrap=True)

    nc.compile()
    return nc


def host_prep(weight, bias):
    w = np.asarray(weight, dtype=np.float32)          # [oc, ic, kh, kw]
    wt = np.transpose(w, (1, 3, 0, 2))                # [ic, kw, oc, kh]

    def pack(top_kh, bot_kh):
        return np.concatenate([wt[:, :, :, top_kh], wt[:, :, :, bot_kh]],
                              axis=0).reshape(2 * IC, 3 * OC).astype(NPBF16)

    weven = pack(0, 1)
    wodd = pack(1, 2)
    wleft = pack(2, 0)
    b = np.asarray(bias, dtype=np.float32).reshape(OC, 1)
    return weven, wodd, wleft, b


def pad_x(x):
    """[N, 64, 224, 224] f32 -> [N, 128, 113, 226] bf16 even/odd rows."""
    n = x.shape[0]
    xb = np.asarray(x, dtype=np.float32).astype(NPBF16)
    pp = np.zeros((n, IC, HP, WP), NPBF16)
    pp[:, :, 1:1 + H, 1:1 + W] = xb
    xeo = np.empty((n, 2 * IC, XR, WP), NPBF16)
    xeo[:, 0:IC] = pp[:, :, 0::2, :]
    xeo[:, IC:2 * IC] = pp[:, :, 1::2, :]
    return xeo


def prep(x, weight, bias):
    """Full inputs -> per-core in_maps list."""
    weven, wodd, wleft, b = host_prep(weight, bias)
    xeo = pad_x(x)
    return [{"xeo": xeo[c * S:(c + 1) * S], "weven": weven, "wodd": wodd,
             "wleft": wleft, "bias": b} for c in range(N_CORES)]


_module_cache = {}


def get_module(repeat=1):
    if repeat not in _module_cache:
        _module_cache[repeat] = build_module(repeat)
    return _module_cache[repeat]


def kernel(x, weight, bias):
    in_maps = prep(x, weight, bias)
    nc = get_module()
    res = run_bass_kernel_spmd(nc, in_maps, core_ids=list(range(N_CORES)))
    return np.concatenate([res.results[c]["out"] for c in range(N_CORES)],
                          axis=0).astype(np.float32)
